# revision 4
# baseline (speedup 1.0000x reference)
"""Trainium2 Bass kernel for nn_MultiHeadAttentionQuantum.

Math simplification (verified vs reference to ~5e-7):
  The per-token quantum feature map RX(x+theta) -> CNOT ring -> <Z_w>
  collapses to products of cosines. With u_w = cos(x_w + theta_w):
      q_0 = u1*u2*...*u7
      q_w = u0*u1*...*uw   (w = 1..7)
  Then per batch: scores = q @ q.T / sqrt(2); attn = softmax(scores);
  out = attn @ q; out' = swapaxes(out,1,2).reshape(S,8);  y = out' @ Wc.T + b.
  The softmax max-subtraction is skipped (|scores| <= 5.7, exp <= 287, safe
  in fp32). Row sums come free as a ones-column in the second matmul.

Sharding: data-parallel over batch: 16 batches -> 8 cores x 2 batches.

Per-core device pipeline (all fp32):
  phase Q (per batch): DMA x (p-major: token s = 32p + t), add theta+pi/2
    per wire via DVE tensor_scalar (per-partition scalar), u = ACT Sin,
    13 strided DVE muls -> q9 [128, T, 9] (col 8 = ones), PE-transpose
    q9 chunks -> qT [9, S] (token chunk-major: col 128t + p).
  phase A (per batch, per 512-token i-block): for each 128-token j-chunk:
    PE scores[j,i] = qT_j^T(8x128) @ qT_i(8x512) -> PSUM; ACT exp over
    3-chunk groups [128,1536] PSUM->SBUF (scale=1/sqrt2 folded in);
    PE outT[9, 512] += q9_j^T @ exp  (accumulated over all j).
    Normalize: copy outT->SBUF, PE-transpose back to token-major [128, 9],
    DVE reciprocal of ones-row sums, DVE scale -> out_sb; DMA -> DRAM scratch.
  phase C (per batch): the reference's swapaxes+reshape+combine becomes:
    y[128m + p, j] = sum_e oscr[8*(128*mt + p) + e, k] * Wc[j, e] + b[j]
    with m = (S/1024)k + mt.  Strided-DMA gather lhsT [9, 128] tiles
    (row 8 = ones for the bias), one PE matmul vs wcb=[Wc.T; b] [9, 8].
"""

import math
import numpy as np

import concourse.bass as bass
import concourse.bacc as bacc
import concourse.tile as tile
from concourse import mybir
from concourse.masks import make_identity
from concourse._compat import with_exitstack

F32 = mybir.dt.float32
F16 = mybir.dt.float16
AF = mybir.ActivationFunctionType
P = 128
E = 8
E9 = 9
IB = 512          # i-block width (tokens per output accumulation block)
JG = 3            # j-chunks per exp group (3 PSUM banks per scores buffer)
INV_SQRT2 = 0.7071067811865476


@with_exitstack
def _body(ctx, tc, x_in, thp, wcb, y, oscr, S, NB):
    nc = tc.nc
    T = S // P                 # token-chunks (tokens per partition)
    NIB = S // IB              # i-blocks per batch
    M4 = S // (P * E)          # row-tiles per combine feature block
    CPI = IB // P              # chunks per i-block (4)

    const = ctx.enter_context(tc.tile_pool(name="const", bufs=1))
    qpool = ctx.enter_context(tc.tile_pool(name="qdata", bufs=1))
    work = ctx.enter_context(tc.tile_pool(name="work", bufs=2))
    expp = ctx.enter_context(tc.tile_pool(name="expp", bufs=3))
    fin = ctx.enter_context(tc.tile_pool(name="fin", bufs=3))
    scps = ctx.enter_context(tc.tile_pool(name="scps", bufs=2, space="PSUM"))
    outps = ctx.enter_context(tc.tile_pool(name="outps", bufs=2, space="PSUM"))

    ident = const.tile([P, P], F32)
    make_identity(nc, ident[:])
    thp_sb = const.tile([P, E], F32)
    nc.sync.dma_start(thp_sb[:], thp[:])
    wcb_sb = const.tile([E9, E], F32)
    nc.sync.dma_start(wcb_sb[:], wcb[:])

    q9 = [qpool.tile([P, T * E9], F32, name=f"q9_{b}") for b in range(NB)]
    qT = [qpool.tile([E9, S], F16, name=f"qT_{b}") for b in range(NB)]
    q9h = [qpool.tile([P, T * E9], F16, name=f"q9h_{b}") for b in range(NB)]
    osb = [qpool.tile([P, T * E], F32, name=f"osb_{b}") for b in range(NB)]

    # ---------------- phase Q: quantum features --------------------------
    for b in range(NB):
        xb = x_in[b].rearrange("(p t) w -> p (t w)", p=P)
        xs = work.tile([P, T * E], F32, tag="xs")
        nc.sync.dma_start(xs[:], xb)
        x3 = xs.rearrange("p (t w) -> p t w", w=E)
        ph = work.tile([P, T * E], F32, tag="ph")
        p3 = ph.rearrange("p (t w) -> p t w", w=E)
        for w in range(E):
            nc.vector.tensor_scalar_add(p3[:, :, w], x3[:, :, w], thp_sb[:, w : w + 1])
        # range-reduce ph mod 2*pi into [-pi, pi] (Sin spline domain):
        # n = round(ph / 2pi) via the fp32 magic-constant trick, ph -= n * 2pi
        MAGIC = 12582912.0  # 1.5 * 2**23
        TWO_PI = 6.283185307179586
        rt = work.tile([P, T * E], F32, tag="rt")
        nc.vector.tensor_scalar(
            rt[:], ph[:], 1.0 / TWO_PI, MAGIC, mybir.AluOpType.mult, mybir.AluOpType.add
        )
        nc.vector.tensor_scalar(
            rt[:], rt[:], MAGIC, -TWO_PI, mybir.AluOpType.subtract, mybir.AluOpType.mult
        )
        nc.vector.tensor_add(ph[:], ph[:], rt[:])
        us = work.tile([P, T * E], F32, tag="us")
        nc.scalar.activation(us[:], ph[:], AF.Sin)
        u3 = us.rearrange("p (t w) -> p t w", w=E)

        q = q9[b]
        nc.vector.memset(q[:], 1.0)
        q3 = q.rearrange("p (t e) -> p t e", e=E9)
        nc.vector.tensor_mul(q3[:, :, 1], u3[:, :, 0], u3[:, :, 1])
        for w in range(2, E):
            nc.vector.tensor_mul(q3[:, :, w], q3[:, :, w - 1], u3[:, :, w])
        nc.vector.tensor_mul(q3[:, :, 0], u3[:, :, 1], u3[:, :, 2])
        for w in range(3, E):
            nc.vector.tensor_mul(q3[:, :, 0], q3[:, :, 0], u3[:, :, w])

        nc.vector.tensor_copy(q9h[b][:], q[:])
        # transpose q9 token-chunks into qT (col 128*t + p)
        for c0 in range(0, T, 4):
            tp = outps.tile([P, IB], F32, tag="X")
            for c in range(4):
                nc.tensor.transpose(
                    tp[0:E9, c * P : (c + 1) * P], q3[:, c0 + c, :], ident[:]
                )
            nc.vector.tensor_copy(qT[b][:, c0 * P : (c0 + 4) * P], tp[0:E9, :])

    # ---------------- phase A: attention ---------------------------------
    for b in range(NB):
        qh3 = q9h[b].rearrange("p (t e) -> p t e", e=E9)
        for ib in range(NIB):
            X = outps.tile([P, IB], F32, tag="X")
            xac = X[0:E9, 0:IB]
            rhsQ = qT[b][0:E, ib * IB : (ib + 1) * IB]
            for g0 in range(0, T, JG):
                gn = min(JG, T - g0)
                sc = scps.tile([P, JG * IB], F32, tag="sc")
                for g in range(gn):
                    tj = g0 + g
                    nc.tensor.matmul(
                        sc[:, g * IB : (g + 1) * IB],
                        qT[b][0:E, tj * P : (tj + 1) * P],
                        rhsQ,
                        start=True,
                        stop=True,
                    )
                ex = expp.tile([P, JG * IB], F16, tag="ex")
                nc.scalar.activation(
                    ex[:, 0 : gn * IB], sc[:, 0 : gn * IB], AF.Exp, scale=INV_SQRT2
                )
                for g in range(gn):
                    tj = g0 + g
                    nc.tensor.matmul(
                        xac,
                        qh3[:, tj, :],
                        ex[:, g * IB : (g + 1) * IB],
                        start=(tj == 0),
                        stop=(tj == T - 1),
                    )
            # normalize + back to token-major
            oT = work.tile([E9, IB], F32, tag="oT")
            nc.vector.tensor_copy(oT[:], xac)
            Y = outps.tile([P, IB], F32, tag="X")
            for c in range(CPI):
                nc.tensor.transpose(
                    Y[:, c * E9 : (c + 1) * E9],
                    oT[0:E9, c * P : (c + 1) * P],
                    ident[0:E9, 0:E9],
                )
            Y3 = Y[:, 0 : CPI * E9].rearrange("p (c e) -> p c e", e=E9)
            rec = work.tile([P, CPI], F32, tag="rec")
            nc.vector.reciprocal(rec[:], Y3[:, :, 8])
            o3 = osb[b].rearrange("p (t w) -> p t w", w=E)
            for c in range(CPI):
                t0 = ib * CPI + c
                nc.vector.tensor_scalar_mul(
                    o3[:, t0, :], Y3[:, c, 0:E], rec[:, c : c + 1]
                )
        nc.sync.dma_start(oscr[b].rearrange("(p t) w -> p (t w)", p=P), osb[b][:])

    # ---------------- phase C: combine + un-shuffle -----------------------
    for b in range(NB):
        osrc = oscr[b].rearrange("(m p e) w -> m e p w", m=M4, p=P, e=E)
        for m in range(S // P):
            k, mt = m // M4, m % M4
            lhs = fin.tile([E9, P], F32, tag="lhs")
            nc.vector.memset(lhs[:], 1.0)
            nc.sync.dma_start(lhs[0:E, :], osrc[mt, :, :, k])
            rp = outps.tile([P, IB], F32, tag="X")
            nc.tensor.matmul(rp[:, 0:E], lhs[:], wcb_sb[:], start=True, stop=True)
            rs = fin.tile([P, E], F32, tag="rs")
            nc.vector.tensor_copy(rs[:], rp[:, 0:E])
            nc.sync.dma_start(y[b][m * P : (m + 1) * P, :], rs[:])


def build_nc(S=4096, NB=2):
    nc = bacc.Bacc(None, target_bir_lowering=False)
    x_in = nc.dram_tensor("x", (NB, S, E), F32, kind="ExternalInput")
    thp = nc.dram_tensor("thp", (P, E), F32, kind="ExternalInput")
    wcb = nc.dram_tensor("wcb", (E9, E), F32, kind="ExternalInput")
    y = nc.dram_tensor("y", (NB, S, E), F32, kind="ExternalOutput")
    oscr = nc.dram_tensor("oscr", (NB, S, E), F32)
    with tile.TileContext(nc) as tc:
        _body(tc, x_in[:], thp[:], wcb[:], y[:], oscr[:], S, NB)
    nc.compile()
    return nc


def host_inputs(theta, w_combine, b_combine):
    thp = np.tile(
        (np.asarray(theta, np.float32) + np.float32(np.pi / 2))[None, :], (P, 1)
    ).astype(np.float32)
    wcb = np.concatenate(
        [np.asarray(w_combine, np.float32).T, np.asarray(b_combine, np.float32)[None]],
        axis=0,
    ).astype(np.float32)
    return thp, wcb


_NC_CACHE = {}


def kernel(x, theta, w_combine, b_combine):
    from concourse.bass_utils import run_bass_kernel_spmd

    x = np.asarray(x, np.float32)
    B, S, _ = x.shape
    NCORES = 8
    NB = B // NCORES
    key = (S, NB)
    if key not in _NC_CACHE:
        _NC_CACHE[key] = build_nc(S=S, NB=NB)
    nc = _NC_CACHE[key]
    thp, wcb = host_inputs(theta, w_combine, b_combine)
    in_maps = [
        {"x": x[c * NB : (c + 1) * NB], "thp": thp, "wcb": wcb} for c in range(NCORES)
    ]
    res = run_bass_kernel_spmd(nc, in_maps, list(range(NCORES))).results
    return np.concatenate([res[c]["y"] for c in range(NCORES)], axis=0)


# revision 7
# speedup vs baseline: 1.6770x; 1.6770x over previous
"""Trainium2 Bass kernel for nn_MultiHeadAttentionQuantum.

Math simplification (verified vs reference to ~5e-7):
  The per-token quantum feature map RX(x+theta) -> CNOT ring -> <Z_w>
  collapses to products of cosines. With u_w = cos(x_w + theta_w):
      q_0 = u1*u2*...*u7
      q_w = u0*u1*...*uw   (w = 1..7)
  Then per batch: scores = q @ q.T / sqrt(2); attn = softmax(scores);
  out = attn @ q; out' = swapaxes(out,1,2).reshape(S,8);  y = out' @ Wc.T + b.
  The softmax max-subtraction is skipped (|scores| <= 5.7, exp <= 287, safe
  in fp32). Row sums come free as a ones-column in the second matmul.

Sharding: data-parallel over batch: 16 batches -> 8 cores x 2 batches.

Per-core device pipeline (all fp32):
  phase Q (per batch): DMA x (p-major: token s = 32p + t), add theta+pi/2
    per wire via DVE tensor_scalar (per-partition scalar), u = ACT Sin,
    13 strided DVE muls -> q9 [128, T, 9] (col 8 = ones), PE-transpose
    q9 chunks -> qT [9, S] (token chunk-major: col 128t + p).
  phase A (per batch, per 512-token i-block): for each 128-token j-chunk:
    PE scores[j,i] = qT_j^T(8x128) @ qT_i(8x512) -> PSUM; ACT exp over
    3-chunk groups [128,1536] PSUM->SBUF (scale=1/sqrt2 folded in);
    PE outT[9, 512] += q9_j^T @ exp  (accumulated over all j).
    Normalize: copy outT->SBUF, PE-transpose back to token-major [128, 9],
    DVE reciprocal of ones-row sums, DVE scale -> out_sb; DMA -> DRAM scratch.
  phase C (per batch): the reference's swapaxes+reshape+combine becomes:
    y[128m + p, j] = sum_e oscr[8*(128*mt + p) + e, k] * Wc[j, e] + b[j]
    with m = (S/1024)k + mt.  Strided-DMA gather lhsT [9, 128] tiles
    (row 8 = ones for the bias), one PE matmul vs wcb=[Wc.T; b] [9, 8].
"""

import math
import numpy as np

import concourse.bass as bass
import concourse.bacc as bacc
import concourse.tile as tile
from concourse import mybir
from concourse.masks import make_identity
from concourse._compat import with_exitstack

F32 = mybir.dt.float32
F16 = mybir.dt.float16
AF = mybir.ActivationFunctionType
P = 128
E = 8
E9 = 9
IB = 512          # i-block width (tokens per output accumulation block)
JG = 3            # j-chunks per exp group (3 PSUM banks per scores buffer)
INV_SQRT2 = 0.7071067811865476


@with_exitstack
def _body(ctx, tc, x_in, thp, wcb, sel, y, oscr, S, NB):
    nc = tc.nc
    T = S // P                 # token-chunks (tokens per partition)
    NIB = S // IB              # i-blocks per batch
    M4 = S // (P * E)          # row-tiles per combine feature block
    CPI = IB // P              # chunks per i-block (4)

    const = ctx.enter_context(tc.tile_pool(name="const", bufs=1))
    qpool = ctx.enter_context(tc.tile_pool(name="qdata", bufs=1))
    work = ctx.enter_context(tc.tile_pool(name="work", bufs=2))
    expp = ctx.enter_context(tc.tile_pool(name="expp", bufs=3))
    fin = ctx.enter_context(tc.tile_pool(name="fin", bufs=3))
    scps = ctx.enter_context(tc.tile_pool(name="scps", bufs=2, space="PSUM"))
    outps = ctx.enter_context(tc.tile_pool(name="outps", bufs=2, space="PSUM"))

    ident = const.tile([P, P], F32)
    make_identity(nc, ident[:])
    thp_sb = const.tile([P, E], F32)
    nc.sync.dma_start(thp_sb[:], thp[:])
    wcb_sb = const.tile([E9, E], F32)
    nc.sync.dma_start(wcb_sb[:], wcb[:])
    sel_sb = const.tile([P, E9], F32)
    nc.sync.dma_start(sel_sb[:], sel[:])

    q9 = [qpool.tile([P, T * E9], F32, name=f"q9_{b}") for b in range(NB)]
    qT = [qpool.tile([P, S], F16, name=f"qT_{b}") for b in range(NB)]
    q9h = [qpool.tile([P, T * E9], F16, name=f"q9h_{b}") for b in range(NB)]
    osb = [qpool.tile([P, T * E], F32, name=f"osb_{b}") for b in range(NB)]

    # ---------------- phase Q: quantum features --------------------------
    for b in range(NB):
        xb = x_in[b].rearrange("(p t) w -> p (t w)", p=P)
        xs = work.tile([P, T * E], F32, tag="xs")
        nc.sync.dma_start(xs[:], xb)
        x3 = xs.rearrange("p (t w) -> p t w", w=E)
        ph = work.tile([P, T * E], F32, tag="ph")
        p3 = ph.rearrange("p (t w) -> p t w", w=E)
        for w in range(E):
            nc.vector.tensor_scalar_add(p3[:, :, w], x3[:, :, w], thp_sb[:, w : w + 1])
        # range-reduce ph mod 2*pi into [-pi, pi] (Sin spline domain):
        # n = round(ph / 2pi) via the fp32 magic-constant trick, ph -= n * 2pi
        MAGIC = 12582912.0  # 1.5 * 2**23
        TWO_PI = 6.283185307179586
        rt = work.tile([P, T * E], F32, tag="rt")
        nc.vector.tensor_scalar(
            rt[:], ph[:], 1.0 / TWO_PI, MAGIC, mybir.AluOpType.mult, mybir.AluOpType.add
        )
        nc.vector.tensor_scalar(
            rt[:], rt[:], MAGIC, -TWO_PI, mybir.AluOpType.subtract, mybir.AluOpType.mult
        )
        nc.vector.tensor_add(ph[:], ph[:], rt[:])
        us = work.tile([P, T * E], F32, tag="us")
        nc.scalar.activation(us[:], ph[:], AF.Sin)
        u3 = us.rearrange("p (t w) -> p t w", w=E)

        q = q9[b]
        nc.vector.memset(q[:], 1.0)
        q3 = q.rearrange("p (t e) -> p t e", e=E9)
        nc.vector.tensor_mul(q3[:, :, 1], u3[:, :, 0], u3[:, :, 1])
        for w in range(2, E):
            nc.vector.tensor_mul(q3[:, :, w], q3[:, :, w - 1], u3[:, :, w])
        nc.vector.tensor_mul(q3[:, :, 0], u3[:, :, 1], u3[:, :, 2])
        for w in range(3, E):
            nc.vector.tensor_mul(q3[:, :, 0], q3[:, :, 0], u3[:, :, w])

        nc.vector.tensor_copy(q9h[b][:], q[:])
        # transpose q9 token-chunks into qT (col 128*t + p)
        for c0 in range(0, T, 4):
            tp = outps.tile([P, IB], F32, tag="X")
            for c in range(4):
                nc.tensor.transpose(
                    tp[0:E9, c * P : (c + 1) * P], q3[:, c0 + c, :], ident[:]
                )
            nc.vector.tensor_copy(qT[b][0:E9, c0 * P : (c0 + 4) * P], tp[0:E9, :])
        for r in range(1, 4):
            nc.sync.dma_start(qT[b][32 * r : 32 * r + E9, :], qT[b][0:E9, :])

    # ---------------- phase A: attention ---------------------------------
    for b in range(NB):
        qh3 = q9h[b].rearrange("p (t e) -> p t e", e=E9)
        for ib in range(NIB):
            X = outps.tile([P, IB], F32, tag="X")
            nc.vector.memset(X[:], 0.0)
            for g0 in range(0, T, JG):
                gn = min(JG, T - g0)
                sc = scps.tile([P, JG * IB], F32, tag="sc")
                for g in range(gn):
                    tj = g0 + g
                    rb = 32 * g
                    nc.tensor.matmul(
                        sc[:, g * IB : (g + 1) * IB],
                        qT[b][rb : rb + E, tj * P : (tj + 1) * P],
                        qT[b][rb : rb + E, ib * IB : (ib + 1) * IB],
                        start=True,
                        stop=True,
                        tile_position=(rb, 0),
                    )
                ex = expp.tile([P, JG * IB], F16, tag="ex")
                nc.scalar.activation(
                    ex[:, 0 : gn * IB], sc[:, 0 : gn * IB], AF.Exp, scale=INV_SQRT2
                )
                for g in range(gn):
                    tj = g0 + g
                    cs = 32 * (tj % 4)
                    nc.tensor.matmul(
                        X[cs : cs + E9, :],
                        qh3[:, tj, :],
                        ex[:, g * IB : (g + 1) * IB],
                        start=(tj == 0),
                        stop=(tj == T - 1),
                        tile_position=(0, cs),
                        skip_group_check=True,
                    )
            # normalize + strip-sum + back to token-major (via sel matmul)
            Xs = work.tile([P, IB], F32, tag="Xs")
            nc.vector.tensor_copy(Xs[:], X[:])
            Y = outps.tile([P, IB], F32, tag="X")
            for c in range(CPI):
                nc.tensor.matmul(
                    Y[:, c * E9 : (c + 1) * E9],
                    Xs[:, c * P : (c + 1) * P],
                    sel_sb[:],
                    start=True,
                    stop=True,
                )
            Y3 = Y[:, 0 : CPI * E9].rearrange("p (c e) -> p c e", e=E9)
            rec = work.tile([P, CPI], F32, tag="rec")
            nc.vector.reciprocal(rec[:], Y3[:, :, 8])
            o3 = osb[b].rearrange("p (t w) -> p t w", w=E)
            for c in range(CPI):
                t0 = ib * CPI + c
                nc.vector.tensor_scalar_mul(
                    o3[:, t0, :], Y3[:, c, 0:E], rec[:, c : c + 1]
                )
        nc.sync.dma_start(oscr[b].rearrange("(p t) w -> p (t w)", p=P), osb[b][:])

    # ---------------- phase C: combine + un-shuffle -----------------------
    for b in range(NB):
        osrc = oscr[b].rearrange("(m p e) w -> m e p w", m=M4, p=P, e=E)
        for m in range(S // P):
            k, mt = m // M4, m % M4
            lhs = fin.tile([E9, P], F32, tag="lhs")
            nc.vector.memset(lhs[:], 1.0)
            nc.sync.dma_start(lhs[0:E, :], osrc[mt, :, :, k])
            rp = outps.tile([P, IB], F32, tag="X")
            nc.tensor.matmul(rp[:, 0:E], lhs[:], wcb_sb[:], start=True, stop=True)
            rs = fin.tile([P, E], F32, tag="rs")
            nc.vector.tensor_copy(rs[:], rp[:, 0:E])
            nc.sync.dma_start(y[b][m * P : (m + 1) * P, :], rs[:])


def build_nc(S=4096, NB=2):
    nc = bacc.Bacc(None, target_bir_lowering=False)
    x_in = nc.dram_tensor("x", (NB, S, E), F32, kind="ExternalInput")
    thp = nc.dram_tensor("thp", (P, E), F32, kind="ExternalInput")
    wcb = nc.dram_tensor("wcb", (E9, E), F32, kind="ExternalInput")
    sel = nc.dram_tensor("sel", (P, E9), F32, kind="ExternalInput")
    y = nc.dram_tensor("y", (NB, S, E), F32, kind="ExternalOutput")
    oscr = nc.dram_tensor("oscr", (NB, S, E), F32)
    with tile.TileContext(nc) as tc:
        _body(tc, x_in[:], thp[:], wcb[:], sel[:], y[:], oscr[:], S, NB)
    nc.compile()
    return nc


def host_inputs(theta, w_combine, b_combine):
    thp = np.tile(
        (np.asarray(theta, np.float32) + np.float32(np.pi / 2))[None, :], (P, 1)
    ).astype(np.float32)
    wcb = np.concatenate(
        [np.asarray(w_combine, np.float32).T, np.asarray(b_combine, np.float32)[None]],
        axis=0,
    ).astype(np.float32)
    sel = np.zeros((P, E9), np.float32)
    for st in range(4):
        for e in range(E9):
            sel[32 * st + e, e] = 1.0
    return thp, wcb, sel


_NC_CACHE = {}


def kernel(x, theta, w_combine, b_combine):
    from concourse.bass_utils import run_bass_kernel_spmd

    x = np.asarray(x, np.float32)
    B, S, _ = x.shape
    NCORES = 8
    NB = B // NCORES
    key = (S, NB)
    if key not in _NC_CACHE:
        _NC_CACHE[key] = build_nc(S=S, NB=NB)
    nc = _NC_CACHE[key]
    thp, wcb, sel = host_inputs(theta, w_combine, b_combine)
    in_maps = [
        {"x": x[c * NB : (c + 1) * NB], "thp": thp, "wcb": wcb, "sel": sel}
        for c in range(NCORES)
    ]
    res = run_bass_kernel_spmd(nc, in_maps, list(range(NCORES))).results
    return np.concatenate([res[c]["y"] for c in range(NCORES)], axis=0)


# revision 10
# speedup vs baseline: 1.9270x; 1.1491x over previous
"""Trainium2 Bass kernel for nn_MultiHeadAttentionQuantum.

Math simplification (verified vs reference to ~5e-7):
  The per-token quantum feature map RX(x+theta) -> CNOT ring -> <Z_w>
  collapses to products of cosines. With u_w = cos(x_w + theta_w):
      q_0 = u1*u2*...*u7
      q_w = u0*u1*...*uw   (w = 1..7)
  Then per batch: scores = q @ q.T / sqrt(2); attn = softmax(scores);
  out = attn @ q; out' = swapaxes(out,1,2).reshape(S,8);  y = out' @ Wc.T + b.
  Softmax max-subtraction is skipped (|scores| <= 5.7, exp <= 287, safe in
  fp32). Row sums come free as a ones-column in the second matmul.

Sharding: data-parallel over batch: 16 batches -> 8 cores x 2 batches.

Per-core device pipeline:
  phase Q (per batch): DMA x p-major (token s = 32p + t), add theta+pi/2 per
    wire (DVE per-partition scalar), range-reduce mod 2pi, u = ACT Sin,
    13 strided DVE muls -> q9 [128, T, 9] fp32 (col 8 = ones) + fp16 copy,
    PE-transpose chunks -> qT [128, S] fp16 with the 8 feature rows
    replicated at partition strips 0/32/64/96 (for row-group packing).
  phase A (per batch, per 512-token i-block):
    scores: 3 row-group-packed K=8 fp16 matmuls per group -> PSUM [128,1536]
    exp:    one ACT instr per group, scale=1/sqrt2, PSUM->SBUF fp16
    accum:  col-group-packed matmuls X[32s:32s+9] += q9_j^T @ exp
            (strip s = chunk%4; strips summed later by the sel matmul)
    normalize (software-pipelined one i-block behind): DVE copy X->SBUF,
    4 matmuls vs sel[128,9] (sums the 4 strips AND transposes to
    token-major), DVE reciprocal of the ones-row, DVE scale -> osb.
  phase C (per batch, overlaps next batch's attention): the reference's
    swapaxes+reshape+combine is y[128m+p, j] = sum_e oscr[8*(128*mt+p)+e, k]
    * Wc[j,e] + b[j] with m = (S/1024)k + mt: one strided gather DMA into
    glh [9, S] (row 8 = ones for the bias), 32 matmuls vs wcb=[Wc.T; b]
    into one PSUM bank, one DVE copy, one strided store DMA.
"""

import numpy as np

import concourse.bass as bass
import concourse.bacc as bacc
import concourse.tile as tile
from concourse import mybir
from concourse.masks import make_identity
from concourse._compat import with_exitstack

F32 = mybir.dt.float32
F16 = mybir.dt.float16
AF = mybir.ActivationFunctionType
P = 128
E = 8
E9 = 9
IB = 512          # i-block width (tokens per output accumulation block)
JG = 3            # j-chunks per exp group (3 PSUM banks per scores buffer)
INV_SQRT2 = 0.7071067811865476


@with_exitstack
def _body(ctx, tc, x_in, thp, wcb, sel, y, oscr, S, NB):
    nc = tc.nc
    T = S // P                 # token-chunks (tokens per partition)
    NIB = S // IB              # i-blocks per batch
    M4 = S // (P * E)          # row-tiles per combine feature block
    CPI = IB // P              # chunks per i-block (4)

    const = ctx.enter_context(tc.tile_pool(name="const", bufs=1))
    qpool = ctx.enter_context(tc.tile_pool(name="qdata", bufs=1))
    work = ctx.enter_context(tc.tile_pool(name="work", bufs=2))
    expp = ctx.enter_context(tc.tile_pool(name="expp", bufs=3))
    scps = ctx.enter_context(tc.tile_pool(name="scps", bufs=2, space="PSUM"))
    outps = ctx.enter_context(tc.tile_pool(name="outps", bufs=2, space="PSUM"))

    ident = const.tile([P, P], F32)
    make_identity(nc, ident[:])
    thp_sb = const.tile([P, E], F32)
    nc.sync.dma_start(thp_sb[:], thp[:])
    wcb_sb = const.tile([E9, E], F32)
    nc.sync.dma_start(wcb_sb[:], wcb[:])
    sel_sb = const.tile([P, E9], F32)
    nc.sync.dma_start(sel_sb[:], sel[:])

    q9 = [qpool.tile([P, T * E9], F32, name=f"q9_{b}") for b in range(NB)]
    q9h = [qpool.tile([P, T * E9], F16, name=f"q9h_{b}") for b in range(NB)]
    qT = [qpool.tile([P, S], F16, name=f"qT_{b}") for b in range(NB)]
    osb = [qpool.tile([P, T * E], F32, name=f"osb_{b}") for b in range(NB)]
    ysb = [qpool.tile([P, T * E], F32, name=f"ysb_{b}") for b in range(NB)]

    # ---------------- phase Q: quantum features --------------------------
    for b in range(NB):
        xb = x_in[b].rearrange("(p t) w -> p (t w)", p=P)
        xs = work.tile([P, T * E], F32, tag="xs")
        nc.sync.dma_start(xs[:], xb)
        x3 = xs.rearrange("p (t w) -> p t w", w=E)
        ph = work.tile([P, T * E], F32, tag="ph")
        p3 = ph.rearrange("p (t w) -> p t w", w=E)
        for w in range(E):
            nc.vector.tensor_scalar_add(p3[:, :, w], x3[:, :, w], thp_sb[:, w : w + 1])
        # range-reduce ph mod 2*pi into [-pi, pi] (Sin spline domain):
        # n = round(ph / 2pi) via the fp32 magic-constant trick, ph -= n * 2pi
        MAGIC = 12582912.0  # 1.5 * 2**23
        TWO_PI = 6.283185307179586
        rt = work.tile([P, T * E], F32, tag="rt")
        nc.vector.tensor_scalar(
            rt[:], ph[:], 1.0 / TWO_PI, MAGIC, mybir.AluOpType.mult, mybir.AluOpType.add
        )
        nc.vector.tensor_scalar(
            rt[:], rt[:], MAGIC, -TWO_PI, mybir.AluOpType.subtract, mybir.AluOpType.mult
        )
        nc.vector.tensor_add(ph[:], ph[:], rt[:])
        us = work.tile([P, T * E], F32, tag="us")
        nc.scalar.activation(us[:], ph[:], AF.Sin)
        u3 = us.rearrange("p (t w) -> p t w", w=E)

        q = q9[b]
        nc.vector.memset(q[:], 1.0)
        q3 = q.rearrange("p (t e) -> p t e", e=E9)
        nc.vector.tensor_mul(q3[:, :, 1], u3[:, :, 0], u3[:, :, 1])
        for w in range(2, E):
            nc.vector.tensor_mul(q3[:, :, w], q3[:, :, w - 1], u3[:, :, w])
        nc.vector.tensor_mul(q3[:, :, 0], u3[:, :, 1], u3[:, :, 2])
        for w in range(3, E):
            nc.vector.tensor_mul(q3[:, :, 0], q3[:, :, 0], u3[:, :, w])

        nc.vector.tensor_copy(q9h[b][:], q[:])
        # transpose q9 token-chunks into qT rows 0:9 (col 128*t + p), then
        # replicate the slice to partition strips 32/64/96 via SBUF DMA
        for c0 in range(0, T, 4):
            tp = outps.tile([P, IB], F32, tag="X")
            for c in range(4):
                nc.tensor.transpose(
                    tp[0:E9, c * P : (c + 1) * P], q3[:, c0 + c, :], ident[:]
                )
            cols = slice(c0 * P, (c0 + 4) * P)
            nc.vector.tensor_copy(qT[b][0:E9, cols], tp[0:E9, :])
            for r in range(1, 4):
                nc.sync.dma_start(qT[b][32 * r : 32 * r + E, cols], qT[b][0:E, cols])

    # ---------------- phases A + C, batch-pipelined -----------------------
    for b in range(NB):
        qh3 = q9h[b].rearrange("p (t e) -> p t e", e=E9)
        o3 = osb[b].rearrange("p (t w) -> p t w", w=E)
        pending = None  # deferred normalize of the previous i-block

        def normalize(X, ib):
            Xs = work.tile([P, IB], F32, tag="Xs")
            nc.vector.tensor_copy(Xs[:], X[:])
            Y = outps.tile([P, IB], F32, tag="X")
            for c in range(CPI):
                nc.tensor.matmul(
                    Y[:, c * E9 : (c + 1) * E9],
                    Xs[:, c * P : (c + 1) * P],
                    sel_sb[:],
                    start=True,
                    stop=True,
                )
            Y3 = Y[:, 0 : CPI * E9].rearrange("p (c e) -> p c e", e=E9)
            rec = work.tile([P, CPI], F32, tag="rec")
            nc.vector.reciprocal(rec[:], Y3[:, :, 8])
            for c in range(CPI):
                nc.vector.tensor_scalar_mul(
                    o3[:, ib * CPI + c, :], Y3[:, c, 0:E], rec[:, c : c + 1]
                )

        for ib in range(NIB):
            X = outps.tile([P, IB], F32, tag="X")
            nc.vector.memset(X[:], 0.0)
            for g0 in range(0, T, JG):
                gn = min(JG, T - g0)
                sc = scps.tile([P, JG * IB], F32, tag="sc")
                for g in range(gn):
                    tj = g0 + g
                    rb = 32 * g
                    nc.tensor.matmul(
                        sc[:, g * IB : (g + 1) * IB],
                        qT[b][rb : rb + E, tj * P : (tj + 1) * P],
                        qT[b][rb : rb + E, ib * IB : (ib + 1) * IB],
                        start=True,
                        stop=True,
                        tile_position=(rb, 0),
                    )
                ex = expp.tile([P, JG * IB], F16, tag="ex")
                nc.scalar.activation(
                    ex[:, 0 : gn * IB], sc[:, 0 : gn * IB], AF.Exp, scale=INV_SQRT2
                )
                for g in range(gn):
                    tj = g0 + g
                    cs = 32 * (tj % 4)
                    nc.tensor.matmul(
                        X[cs : cs + E9, :],
                        qh3[:, tj, :],
                        ex[:, g * IB : (g + 1) * IB],
                        start=(tj == 0),
                        stop=(tj == T - 1),
                        tile_position=(0, cs),
                        skip_group_check=True,
                    )
                if g0 == 0 and pending is not None:
                    normalize(*pending)
                    pending = None
            pending = (X, ib)
        normalize(*pending)
        nc.sync.dma_start(oscr[b].rearrange("(p t) w -> p (t w)", p=P), osb[b][:])

        # ---- phase C: combine + un-shuffle (overlaps next batch's A) ----
        glh = qpool.tile([E9, M4 * P * E], F32, name=f"glh_{b}")
        nc.vector.memset(glh[:], 1.0)
        glh4 = glh.rearrange("p (mt pp k) -> p mt pp k", pp=P, k=E)
        nc.sync.dma_start(
            glh4[0:E],
            oscr[b].rearrange("(mt pp e) w -> e mt pp w", e=E, pp=P),
        )
        rp = outps.tile([P, IB], F32, tag="X")
        for m in range(S // P):
            k, mt = m // M4, m % M4
            nc.tensor.matmul(
                rp[:, m * E : (m + 1) * E],
                glh4[:, mt, :, k],
                wcb_sb[:],
                start=True,
                stop=True,
            )
        nc.vector.tensor_copy(ysb[b][:], rp[:, 0 : T * E])
        nc.sync.dma_start(
            y[b].rearrange("(m pp) j -> pp m j", pp=P),
            ysb[b].rearrange("p (m j) -> p m j", j=E),
        )


def build_nc(S=4096, NB=2):
    nc = bacc.Bacc(None, target_bir_lowering=False)
    x_in = nc.dram_tensor("x", (NB, S, E), F32, kind="ExternalInput")
    thp = nc.dram_tensor("thp", (P, E), F32, kind="ExternalInput")
    wcb = nc.dram_tensor("wcb", (E9, E), F32, kind="ExternalInput")
    sel = nc.dram_tensor("sel", (P, E9), F32, kind="ExternalInput")
    y = nc.dram_tensor("y", (NB, S, E), F32, kind="ExternalOutput")
    oscr = nc.dram_tensor("oscr", (NB, S, E), F32)
    with tile.TileContext(nc) as tc:
        _body(tc, x_in[:], thp[:], wcb[:], sel[:], y[:], oscr[:], S, NB)
    nc.compile()
    return nc


def host_inputs(theta, w_combine, b_combine):
    thp = np.tile(
        (np.asarray(theta, np.float32) + np.float32(np.pi / 2))[None, :], (P, 1)
    ).astype(np.float32)
    wcb = np.concatenate(
        [np.asarray(w_combine, np.float32).T, np.asarray(b_combine, np.float32)[None]],
        axis=0,
    ).astype(np.float32)
    sel = np.zeros((P, E9), np.float32)
    for st in range(4):
        for e in range(E9):
            sel[32 * st + e, e] = 1.0
    return thp, wcb, sel


_NC_CACHE = {}


def kernel(x, theta, w_combine, b_combine):
    from concourse.bass_utils import run_bass_kernel_spmd

    x = np.asarray(x, np.float32)
    B, S, _ = x.shape
    NCORES = 8
    NB = B // NCORES
    key = (S, NB)
    if key not in _NC_CACHE:
        _NC_CACHE[key] = build_nc(S=S, NB=NB)
    nc = _NC_CACHE[key]
    thp, wcb, sel = host_inputs(theta, w_combine, b_combine)
    in_maps = [
        {"x": x[c * NB : (c + 1) * NB], "thp": thp, "wcb": wcb, "sel": sel}
        for c in range(NCORES)
    ]
    res = run_bass_kernel_spmd(nc, in_maps, list(range(NCORES))).results
    return np.concatenate([res[c]["y"] for c in range(NCORES)], axis=0)


# revision 15
# speedup vs baseline: 1.9963x; 1.0359x over previous
"""Trainium2 Bass kernel for nn_MultiHeadAttentionQuantum.

Math simplification (verified vs reference to ~5e-7):
  The per-token quantum feature map RX(x+theta) -> CNOT ring -> <Z_w>
  collapses to products of cosines. With u_w = cos(x_w + theta_w):
      q_0 = u1*u2*...*u7
      q_w = u0*u1*...*uw   (w = 1..7)
  Then per batch: scores = q @ q.T / sqrt(2); attn = softmax(scores);
  out = attn @ q; out' = swapaxes(out,1,2).reshape(S,8);  y = out' @ Wc.T + b.
  Softmax max-subtraction is skipped (|scores| <= 5.7, exp <= 287, safe in
  fp32). Row sums come free as a ones-column in the second matmul.

Sharding: data-parallel over batch: 16 batches -> 8 cores x 2 batches.

Per-core device pipeline:
  phase Q (per batch): DMA x p-major (token s = 32p + t), add theta+pi/2 per
    wire (DVE per-partition scalar), range-reduce mod 2pi, u = ACT Sin,
    13 strided DVE muls -> q9 [128, T, 9] fp32 (col 8 = ones) + fp16 copy,
    PE-transpose chunks -> qT [128, S] fp16 with the 8 feature rows
    replicated at partition strips 0/32/64/96 (for row-group packing).
  phase A (per batch, per 512-token i-block):
    scores: 3 row-group-packed K=8 fp16 matmuls per group -> PSUM [128,1536]
    exp:    one ACT instr per group, scale=1/sqrt2, PSUM->SBUF fp16
    accum:  col-group-packed matmuls X[32s:32s+9] += q9_j^T @ exp
            (strip s = chunk%4; strips summed later by the sel matmul)
    normalize (software-pipelined one i-block behind): DVE copy X->SBUF,
    4 matmuls vs sel[128,9] (sums the 4 strips AND transposes to
    token-major), DVE reciprocal of the ones-row, DVE scale -> osb.
  phase C (per batch, overlaps next batch's attention): the reference's
    swapaxes+reshape+combine is y[128m+p, j] = sum_e oscr[8*(128*mt+p)+e, k]
    * Wc[j,e] + b[j] with m = (S/1024)k + mt: one strided gather DMA into
    glh [9, S] (row 8 = ones for the bias), 32 matmuls vs wcb=[Wc.T; b]
    into one PSUM bank, one DVE copy, one strided store DMA.
"""

import numpy as np

import concourse.bass as bass
import concourse.bacc as bacc
import concourse.tile as tile
from concourse import mybir
from concourse.masks import make_identity
from concourse._compat import with_exitstack

F32 = mybir.dt.float32
F16 = mybir.dt.float16
AF = mybir.ActivationFunctionType
P = 128
E = 8
E9 = 9
IB = 512          # i-block width (tokens per output accumulation block)
JG = 3            # j-chunks per exp group (3 PSUM banks per scores buffer)
INV_SQRT2 = 0.7071067811865476


@with_exitstack
def _body(ctx, tc, x_in, thp, wcb, sel, y, oscr, S, NB):
    nc = tc.nc
    T = S // P                 # token-chunks (tokens per partition)
    NIB = S // IB              # i-blocks per batch
    M4 = S // (P * E)          # row-tiles per combine feature block
    CPI = IB // P              # chunks per i-block (4)

    const = ctx.enter_context(tc.tile_pool(name="const", bufs=1))
    qpool = ctx.enter_context(tc.tile_pool(name="qdata", bufs=1))
    work = ctx.enter_context(tc.tile_pool(name="work", bufs=2))
    expp = ctx.enter_context(tc.tile_pool(name="expp", bufs=3))
    scps = ctx.enter_context(tc.tile_pool(name="scps", bufs=2, space="PSUM"))
    outps = ctx.enter_context(tc.tile_pool(name="outps", bufs=2, space="PSUM"))

    ident = const.tile([P, P], F32)
    make_identity(nc, ident[:])
    thp_sb = const.tile([P, E], F32)
    nc.sync.dma_start(thp_sb[:], thp[:])
    wcb_sb = const.tile([P, E], F32)
    nc.sync.dma_start(wcb_sb[:], wcb[:])
    sel_sb = const.tile([P, E9], F32)
    nc.sync.dma_start(sel_sb[:], sel[:])

    q9 = [qpool.tile([P, T * E9], F32, name=f"q9_{b}") for b in range(NB)]
    q9h = [qpool.tile([P, T * E9], F16, name=f"q9h_{b}") for b in range(NB)]
    qT = [qpool.tile([P, S], F16, name=f"qT_{b}") for b in range(NB)]
    osb = [qpool.tile([P, T * E], F32, name=f"osb_{b}") for b in range(NB)]
    ysb = [qpool.tile([P, T * E], F32, name=f"ysb_{b}") for b in range(NB)]

    # ---------------- phase Q: quantum features --------------------------
    for b in range(NB):
        xb = x_in[b].rearrange("(p t) w -> p (t w)", p=P)
        xs = work.tile([P, T * E], F32, tag="xs")
        nc.sync.dma_start(xs[:], xb)
        x3 = xs.rearrange("p (t w) -> p t w", w=E)
        ph = work.tile([P, T * E], F32, tag="ph")
        p3 = ph.rearrange("p (t w) -> p t w", w=E)
        for w in range(E):
            nc.vector.tensor_scalar_add(p3[:, :, w], x3[:, :, w], thp_sb[:, w : w + 1])
        # range-reduce ph mod 2*pi into [-pi, pi] (Sin spline domain):
        # n = round(ph / 2pi) via the fp32 magic-constant trick, ph -= n * 2pi
        MAGIC = 12582912.0  # 1.5 * 2**23
        TWO_PI = 6.283185307179586
        rt = work.tile([P, T * E], F32, tag="rt")
        nc.vector.tensor_scalar(
            rt[:], ph[:], 1.0 / TWO_PI, MAGIC, mybir.AluOpType.mult, mybir.AluOpType.add
        )
        nc.vector.tensor_scalar(
            rt[:], rt[:], MAGIC, -TWO_PI, mybir.AluOpType.subtract, mybir.AluOpType.mult
        )
        nc.vector.tensor_add(ph[:], ph[:], rt[:])
        us = work.tile([P, T * E], F32, tag="us")
        nc.scalar.activation(us[:], ph[:], AF.Sin)
        u3 = us.rearrange("p (t w) -> p t w", w=E)

        q = q9[b]
        nc.vector.memset(q[:], 1.0)
        q3 = q.rearrange("p (t e) -> p t e", e=E9)
        nc.vector.tensor_mul(q3[:, :, 1], u3[:, :, 0], u3[:, :, 1])
        for w in range(2, E):
            nc.vector.tensor_mul(q3[:, :, w], q3[:, :, w - 1], u3[:, :, w])
        nc.vector.tensor_mul(q3[:, :, 0], u3[:, :, 1], u3[:, :, 2])
        for w in range(3, E):
            nc.vector.tensor_mul(q3[:, :, 0], q3[:, :, 0], u3[:, :, w])

        nc.vector.tensor_copy(q9h[b][:], q[:])
        # transpose q9 token-chunks into qT rows 0:9 (col 128*t + p), then
        # replicate the slice to partition strips 32/64/96 via SBUF DMA
        for c0 in range(0, T, 4):
            tp = outps.tile([P, IB], F32, tag="X")
            for c in range(4):
                nc.tensor.transpose(
                    tp[0:E9, c * P : (c + 1) * P], q3[:, c0 + c, :], ident[:]
                )
            cols = slice(c0 * P, (c0 + 4) * P)
            nc.vector.tensor_copy(qT[b][0:E9, cols], tp[0:E9, :])
            for r in range(1, 4):
                nc.sync.dma_start(qT[b][32 * r : 32 * r + E, cols], qT[b][0:E, cols])

    # ---------------- phases A + C, batch-pipelined -----------------------
    def combine(b):
        # phase C: one gather DMA (row 8 stays ones for the bias),
        # replicate to row strips, 4x row-group-packed matmuls vs wcb.
        glh = qpool.tile([P, M4 * P * E], F32, name=f"glh_{b}")
        nc.vector.memset(glh[:], 1.0)
        glh4 = glh.rearrange("p (mt pp k) -> p mt pp k", pp=P, k=E)
        nc.sync.dma_start(
            glh4[0:E],
            oscr[b].rearrange("(mt pp e) w -> e mt pp w", e=E, pp=P),
        )
        for r in range(1, 4):
            nc.sync.dma_start(glh[32 * r : 32 * r + E, :], glh[0:E, :])
        rp = outps.tile([P, IB], F32, tag="X")
        for m in range(S // P):
            k, mt = m // M4, m % M4
            nc.tensor.matmul(
                rp[:, m * E : (m + 1) * E],
                glh4[0:E9, mt, :, k],
                wcb_sb[0:E9, :],
                start=True,
                stop=True,
            )
        nc.vector.tensor_copy(ysb[b][:], rp[:, 0 : T * E])
        nc.sync.dma_start(
            y[b].rearrange("(m pp) j -> pp m j", pp=P),
            ysb[b].rearrange("p (m j) -> p m j", j=E),
        )

    pending_combine = None
    for b in range(NB):
        qh3 = q9h[b].rearrange("p (t e) -> p t e", e=E9)
        o3 = osb[b].rearrange("p (t w) -> p t w", w=E)
        pending = None  # deferred normalize of the previous i-block

        def normalize(X, ib):
            Xs = work.tile([P, IB], F32, tag="Xs")
            nc.vector.tensor_copy(Xs[:], X[:])
            Y = outps.tile([P, IB], F32, tag="X")
            for c in range(CPI):
                nc.tensor.matmul(
                    Y[:, c * E9 : (c + 1) * E9],
                    Xs[:, c * P : (c + 1) * P],
                    sel_sb[:],
                    start=True,
                    stop=True,
                )
            Y3 = Y[:, 0 : CPI * E9].rearrange("p (c e) -> p c e", e=E9)
            rec = work.tile([P, CPI], F32, tag="rec")
            nc.vector.reciprocal(rec[:], Y3[:, :, 8])
            for c in range(CPI):
                nc.vector.tensor_scalar_mul(
                    o3[:, ib * CPI + c, :], Y3[:, c, 0:E], rec[:, c : c + 1]
                )

        for ib in range(NIB):
            X = outps.tile([P, IB], F32, tag="X")
            nc.vector.memset(X[:], 0.0)
            for g0 in range(0, T, JG):
                gn = min(JG, T - g0)
                sc = scps.tile([P, JG * IB], F32, tag="sc")
                for g in range(gn):
                    tj = g0 + g
                    rb = 32 * g
                    nc.tensor.matmul(
                        sc[:, g * IB : (g + 1) * IB],
                        qT[b][rb : rb + E, tj * P : (tj + 1) * P],
                        qT[b][rb : rb + E, ib * IB : (ib + 1) * IB],
                        start=True,
                        stop=True,
                        tile_position=(rb, 0),
                    )
                ex = expp.tile([P, JG * IB], F16, tag="ex")
                nc.scalar.activation(
                    ex[:, 0 : gn * IB], sc[:, 0 : gn * IB], AF.Exp, scale=INV_SQRT2
                )
                for g in range(gn):
                    tj = g0 + g
                    cs = 32 * (tj % 4)
                    nc.tensor.matmul(
                        X[cs : cs + E9, :],
                        qh3[:, tj, :],
                        ex[:, g * IB : (g + 1) * IB],
                        start=(tj == 0),
                        stop=(tj == T - 1),
                        tile_position=(0, cs),
                        skip_group_check=True,
                    )
                if g0 == 0 and pending is not None:
                    normalize(*pending)
                    pending = None
                if g0 == JG and ib == 1 and pending_combine is not None:
                    combine(pending_combine)
                    pending_combine = None
            pending = (X, ib)
        normalize(*pending)
        nc.sync.dma_start(oscr[b].rearrange("(p t) w -> p (t w)", p=P), osb[b][:])
        pending_combine = b
    combine(pending_combine)


def build_nc(S=4096, NB=2):
    nc = bacc.Bacc(None, target_bir_lowering=False)
    x_in = nc.dram_tensor("x", (NB, S, E), F32, kind="ExternalInput")
    thp = nc.dram_tensor("thp", (P, E), F32, kind="ExternalInput")
    wcb = nc.dram_tensor("wcb", (P, E), F32, kind="ExternalInput")
    sel = nc.dram_tensor("sel", (P, E9), F32, kind="ExternalInput")
    y = nc.dram_tensor("y", (NB, S, E), F32, kind="ExternalOutput")
    oscr = nc.dram_tensor("oscr", (NB, S, E), F32)
    with tile.TileContext(nc) as tc:
        _body(tc, x_in[:], thp[:], wcb[:], sel[:], y[:], oscr[:], S, NB)
    nc.compile()
    return nc


def host_inputs(theta, w_combine, b_combine):
    thp = np.tile(
        (np.asarray(theta, np.float32) + np.float32(np.pi / 2))[None, :], (P, 1)
    ).astype(np.float32)
    wcb9 = np.concatenate(
        [np.asarray(w_combine, np.float32).T, np.asarray(b_combine, np.float32)[None]],
        axis=0,
    ).astype(np.float32)
    wcb = np.zeros((P, E), np.float32)
    for st in range(4):
        wcb[32 * st : 32 * st + E9] = wcb9
    sel = np.zeros((P, E9), np.float32)
    for st in range(4):
        for e in range(E9):
            sel[32 * st + e, e] = 1.0
    return thp, wcb, sel


_NC_CACHE = {}


def kernel(x, theta, w_combine, b_combine):
    from concourse.bass_utils import run_bass_kernel_spmd

    x = np.asarray(x, np.float32)
    B, S, _ = x.shape
    NCORES = 8
    NB = B // NCORES
    key = (S, NB)
    if key not in _NC_CACHE:
        _NC_CACHE[key] = build_nc(S=S, NB=NB)
    nc = _NC_CACHE[key]
    thp, wcb, sel = host_inputs(theta, w_combine, b_combine)
    in_maps = [
        {"x": x[c * NB : (c + 1) * NB], "thp": thp, "wcb": wcb, "sel": sel}
        for c in range(NCORES)
    ]
    res = run_bass_kernel_spmd(nc, in_maps, list(range(NCORES))).results
    return np.concatenate([res[c]["y"] for c in range(NCORES)], axis=0)


# revision 16
# speedup vs baseline: 1.9963x; 1.0000x over previous
"""Trainium2 Bass kernel for nn_MultiHeadAttentionQuantum.

Math simplification (verified vs reference to ~5e-7):
  The per-token quantum feature map RX(x+theta) -> CNOT ring -> <Z_w>
  collapses to products of cosines. With u_w = cos(x_w + theta_w):
      q_0 = u1*u2*...*u7
      q_w = u0*u1*...*uw   (w = 1..7)
  Then per batch: scores = q @ q.T / sqrt(2); attn = softmax(scores);
  out = attn @ q; out' = swapaxes(out,1,2).reshape(S,8);  y = out' @ Wc.T + b.
  Softmax max-subtraction is skipped (|scores| <= 5.7, exp <= 287, safe in
  fp32). Row sums come free as a ones-column in the second matmul.

Sharding: data-parallel over batch: 16 batches -> 8 cores x 2 batches.

Per-core device pipeline:
  phase Q (per batch): DMA x p-major (token s = 32p + t), add theta+pi/2 per
    wire (DVE per-partition scalar), range-reduce mod 2pi, u = ACT Sin,
    13 strided DVE muls -> q9 [128, T, 9] fp32 (col 8 = ones) + fp16 copy,
    PE-transpose chunks -> qT [128, S] fp16 with the 8 feature rows
    replicated at partition strips 0/32/64/96 (for row-group packing).
  phase A (per batch, per 512-token i-block):
    scores: 3 row-group-packed K=8 fp16 matmuls per group -> PSUM [128,1536]
    exp:    one ACT instr per group, scale=1/sqrt2, PSUM->SBUF fp16
    accum:  col-group-packed matmuls X[32s:32s+9] += q9_j^T @ exp
            (strip s = chunk%4; strips summed later by the sel matmul)
    normalize (software-pipelined one i-block behind): DVE copy X->SBUF,
    4 matmuls vs sel[128,9] (sums the 4 strips AND transposes to
    token-major), DVE reciprocal of the ones-row, DVE scale -> osb.
  phase C (per batch, overlaps next batch's attention): the reference's
    swapaxes+reshape+combine is y[128m+p, j] = sum_e oscr[8*(128*mt+p)+e, k]
    * Wc[j,e] + b[j] with m = (S/1024)k + mt: one strided gather DMA into
    glh [9, S] (row 8 = ones for the bias), 32 matmuls vs wcb=[Wc.T; b]
    into one PSUM bank, one DVE copy, one strided store DMA.
"""

import numpy as np

import concourse.bass as bass
import concourse.bacc as bacc
import concourse.tile as tile
from concourse import mybir
from concourse.masks import make_identity
from concourse._compat import with_exitstack

F32 = mybir.dt.float32
F16 = mybir.dt.float16
AF = mybir.ActivationFunctionType
P = 128
E = 8
E9 = 9
IB = 512          # i-block width (tokens per output accumulation block)
JG = 3            # j-chunks per exp group (3 PSUM banks per scores buffer)
INV_SQRT2 = 0.7071067811865476


@with_exitstack
def _body(ctx, tc, x_in, thp, wcb, sel, y, oscr, S, NB):
    nc = tc.nc
    T = S // P                 # token-chunks (tokens per partition)
    NIB = S // IB              # i-blocks per batch
    M4 = S // (P * E)          # row-tiles per combine feature block
    CPI = IB // P              # chunks per i-block (4)

    const = ctx.enter_context(tc.tile_pool(name="const", bufs=1))
    qpool = ctx.enter_context(tc.tile_pool(name="qdata", bufs=1))
    work = ctx.enter_context(tc.tile_pool(name="work", bufs=2))
    expp = ctx.enter_context(tc.tile_pool(name="expp", bufs=3))
    scps = ctx.enter_context(tc.tile_pool(name="scps", bufs=2, space="PSUM"))
    outps = ctx.enter_context(tc.tile_pool(name="outps", bufs=2, space="PSUM"))

    ident = const.tile([P, P], F32)
    make_identity(nc, ident[:])
    thp_sb = const.tile([P, E], F32)
    nc.sync.dma_start(thp_sb[:], thp[:])
    wcb_sb = const.tile([P, E], F32)
    nc.sync.dma_start(wcb_sb[:], wcb[:])
    sel_sb = const.tile([P, E9], F32)
    nc.sync.dma_start(sel_sb[:], sel[:])

    q9 = [qpool.tile([P, T * E9], F32, name=f"q9_{b}") for b in range(NB)]
    q9h = [qpool.tile([P, T * E9], F16, name=f"q9h_{b}") for b in range(NB)]
    qT = [qpool.tile([P, S], F16, name=f"qT_{b}") for b in range(NB)]
    osb = [qpool.tile([P, T * E], F32, name=f"osb_{b}") for b in range(NB)]
    ysb = [qpool.tile([P, T * E], F32, name=f"ysb_{b}") for b in range(NB)]

    # ---------------- phase Q: quantum features --------------------------
    for b in range(NB):
        xb = x_in[b].rearrange("(p t) w -> p (t w)", p=P)
        xs = work.tile([P, T * E], F32, tag="xs")
        nc.sync.dma_start(xs[:], xb)
        x3 = xs.rearrange("p (t w) -> p t w", w=E)
        ph = work.tile([P, T * E], F32, tag="ph")
        p3 = ph.rearrange("p (t w) -> p t w", w=E)
        for w in range(E):
            nc.vector.tensor_scalar_add(p3[:, :, w], x3[:, :, w], thp_sb[:, w : w + 1])
        # range-reduce ph mod 2*pi into [-pi, pi] (Sin spline domain):
        # n = round(ph / 2pi) via the fp32 magic-constant trick, ph -= n * 2pi
        MAGIC = 12582912.0  # 1.5 * 2**23
        TWO_PI = 6.283185307179586
        rt = work.tile([P, T * E], F32, tag="rt")
        nc.vector.tensor_scalar(
            rt[:], ph[:], 1.0 / TWO_PI, MAGIC, mybir.AluOpType.mult, mybir.AluOpType.add
        )
        nc.vector.tensor_scalar(
            rt[:], rt[:], MAGIC, -TWO_PI, mybir.AluOpType.subtract, mybir.AluOpType.mult
        )
        nc.vector.tensor_add(ph[:], ph[:], rt[:])
        us = work.tile([P, T * E], F32, tag="us")
        nc.scalar.activation(us[:], ph[:], AF.Sin)
        u3 = us.rearrange("p (t w) -> p t w", w=E)

        q = q9[b]
        nc.vector.memset(q[:], 1.0)
        q3 = q.rearrange("p (t e) -> p t e", e=E9)
        nc.vector.tensor_mul(q3[:, :, 1], u3[:, :, 0], u3[:, :, 1])
        for w in range(2, E):
            nc.vector.tensor_mul(q3[:, :, w], q3[:, :, w - 1], u3[:, :, w])
        nc.vector.tensor_mul(q3[:, :, 0], u3[:, :, 1], u3[:, :, 2])
        for w in range(3, E):
            nc.vector.tensor_mul(q3[:, :, 0], q3[:, :, 0], u3[:, :, w])

        nc.vector.tensor_copy(q9h[b][:], q[:])
        # transpose q9 token-chunks into qT rows 0:9 (col 128*t + p), then
        # replicate the slice to partition strips 32/64/96 via SBUF DMA
        for c0 in range(0, T, 4):
            tp = outps.tile([P, IB], F32, tag="X")
            for c in range(4):
                nc.tensor.transpose(
                    tp[0:E9, c * P : (c + 1) * P], q3[:, c0 + c, :], ident[:]
                )
            cols = slice(c0 * P, (c0 + 4) * P)
            nc.vector.tensor_copy(qT[b][0:E9, cols], tp[0:E9, :])
            for r in range(1, 4):
                nc.sync.dma_start(qT[b][32 * r : 32 * r + E, cols], qT[b][0:E, cols])

    # ---------------- phases A + C, batch-pipelined -----------------------
    def combine(b):
        # phase C: one gather DMA (row 8 stays ones for the bias),
        # replicate to row strips, 4x row-group-packed matmuls vs wcb.
        glh = qpool.tile([P, M4 * P * E], F32, name=f"glh_{b}")
        nc.vector.memset(glh[:], 1.0)
        glh4 = glh.rearrange("p (mt pp k) -> p mt pp k", pp=P, k=E)
        nc.sync.dma_start(
            glh4[0:E],
            oscr[b].rearrange("(mt pp e) w -> e mt pp w", e=E, pp=P),
        )
        for r in range(1, 4):
            nc.sync.dma_start(glh[32 * r : 32 * r + E, :], glh[0:E, :])
        # 3x row-group-packed matmuls; concurrent outputs go to DIFFERENT
        # PSUM banks (same-bank concurrent drains are fatal on HW)
        rp = scps.tile([P, JG * IB], F32, tag="sc")
        for m in range(S // P):
            k, mt = m // M4, m % M4
            r, c = m % 3, m // 3
            nc.tensor.matmul(
                rp[:, r * IB + c * E : r * IB + (c + 1) * E],
                glh4[32 * r : 32 * r + E9, mt, :, k],
                wcb_sb[32 * r : 32 * r + E9, :],
                start=True,
                stop=True,
                tile_position=(32 * r, 0),
            )
        for m in range(S // P):
            r, c = m % 3, m // 3
            nc.vector.tensor_copy(
                ysb[b][:, m * E : (m + 1) * E],
                rp[:, r * IB + c * E : r * IB + (c + 1) * E],
            )
        nc.sync.dma_start(
            y[b].rearrange("(m pp) j -> pp m j", pp=P),
            ysb[b].rearrange("p (m j) -> p m j", j=E),
        )

    pending_combine = None
    for b in range(NB):
        qh3 = q9h[b].rearrange("p (t e) -> p t e", e=E9)
        o3 = osb[b].rearrange("p (t w) -> p t w", w=E)
        pending = None  # deferred normalize of the previous i-block

        def normalize(X, ib):
            Xs = work.tile([P, IB], F32, tag="Xs")
            nc.vector.tensor_copy(Xs[:], X[:])
            Y = outps.tile([P, IB], F32, tag="X")
            for c in range(CPI):
                nc.tensor.matmul(
                    Y[:, c * E9 : (c + 1) * E9],
                    Xs[:, c * P : (c + 1) * P],
                    sel_sb[:],
                    start=True,
                    stop=True,
                )
            Y3 = Y[:, 0 : CPI * E9].rearrange("p (c e) -> p c e", e=E9)
            rec = work.tile([P, CPI], F32, tag="rec")
            nc.vector.reciprocal(rec[:], Y3[:, :, 8])
            for c in range(CPI):
                nc.vector.tensor_scalar_mul(
                    o3[:, ib * CPI + c, :], Y3[:, c, 0:E], rec[:, c : c + 1]
                )

        for ib in range(NIB):
            X = outps.tile([P, IB], F32, tag="X")
            nc.vector.memset(X[:], 0.0)
            for g0 in range(0, T, JG):
                gn = min(JG, T - g0)
                sc = scps.tile([P, JG * IB], F32, tag="sc")
                for g in range(gn):
                    tj = g0 + g
                    rb = 32 * g
                    nc.tensor.matmul(
                        sc[:, g * IB : (g + 1) * IB],
                        qT[b][rb : rb + E, tj * P : (tj + 1) * P],
                        qT[b][rb : rb + E, ib * IB : (ib + 1) * IB],
                        start=True,
                        stop=True,
                        tile_position=(rb, 0),
                    )
                ex = expp.tile([P, JG * IB], F16, tag="ex")
                nc.scalar.activation(
                    ex[:, 0 : gn * IB], sc[:, 0 : gn * IB], AF.Exp, scale=INV_SQRT2
                )
                for g in range(gn):
                    tj = g0 + g
                    cs = 32 * (tj % 4)
                    nc.tensor.matmul(
                        X[cs : cs + E9, :],
                        qh3[:, tj, :],
                        ex[:, g * IB : (g + 1) * IB],
                        start=(tj == 0),
                        stop=(tj == T - 1),
                        tile_position=(0, cs),
                        skip_group_check=True,
                    )
                if g0 == 0 and pending is not None:
                    normalize(*pending)
                    pending = None
                if g0 == JG and ib == 1 and pending_combine is not None:
                    combine(pending_combine)
                    pending_combine = None
            pending = (X, ib)
        normalize(*pending)
        nc.sync.dma_start(oscr[b].rearrange("(p t) w -> p (t w)", p=P), osb[b][:])
        pending_combine = b
    combine(pending_combine)


def build_nc(S=4096, NB=2):
    nc = bacc.Bacc(None, target_bir_lowering=False)
    x_in = nc.dram_tensor("x", (NB, S, E), F32, kind="ExternalInput")
    thp = nc.dram_tensor("thp", (P, E), F32, kind="ExternalInput")
    wcb = nc.dram_tensor("wcb", (P, E), F32, kind="ExternalInput")
    sel = nc.dram_tensor("sel", (P, E9), F32, kind="ExternalInput")
    y = nc.dram_tensor("y", (NB, S, E), F32, kind="ExternalOutput")
    oscr = nc.dram_tensor("oscr", (NB, S, E), F32)
    with tile.TileContext(nc) as tc:
        _body(tc, x_in[:], thp[:], wcb[:], sel[:], y[:], oscr[:], S, NB)
    nc.compile()
    return nc


def host_inputs(theta, w_combine, b_combine):
    thp = np.tile(
        (np.asarray(theta, np.float32) + np.float32(np.pi / 2))[None, :], (P, 1)
    ).astype(np.float32)
    wcb9 = np.concatenate(
        [np.asarray(w_combine, np.float32).T, np.asarray(b_combine, np.float32)[None]],
        axis=0,
    ).astype(np.float32)
    wcb = np.zeros((P, E), np.float32)
    for st in range(4):
        wcb[32 * st : 32 * st + E9] = wcb9
    sel = np.zeros((P, E9), np.float32)
    for st in range(4):
        for e in range(E9):
            sel[32 * st + e, e] = 1.0
    return thp, wcb, sel


_NC_CACHE = {}


def kernel(x, theta, w_combine, b_combine):
    from concourse.bass_utils import run_bass_kernel_spmd

    x = np.asarray(x, np.float32)
    B, S, _ = x.shape
    NCORES = 8
    NB = B // NCORES
    key = (S, NB)
    if key not in _NC_CACHE:
        _NC_CACHE[key] = build_nc(S=S, NB=NB)
    nc = _NC_CACHE[key]
    thp, wcb, sel = host_inputs(theta, w_combine, b_combine)
    in_maps = [
        {"x": x[c * NB : (c + 1) * NB], "thp": thp, "wcb": wcb, "sel": sel}
        for c in range(NCORES)
    ]
    res = run_bass_kernel_spmd(nc, in_maps, list(range(NCORES))).results
    return np.concatenate([res[c]["y"] for c in range(NCORES)], axis=0)


# revision 17
# speedup vs baseline: 2.0714x; 1.0376x over previous
"""Trainium2 Bass kernel for nn_MultiHeadAttentionQuantum.

Math simplification (verified vs reference to ~5e-7):
  The per-token quantum feature map RX(x+theta) -> CNOT ring -> <Z_w>
  collapses to products of cosines. With u_w = cos(x_w + theta_w):
      q_0 = u1*u2*...*u7
      q_w = u0*u1*...*uw   (w = 1..7)
  Then per batch: scores = q @ q.T / sqrt(2); attn = softmax(scores);
  out = attn @ q; out' = swapaxes(out,1,2).reshape(S,8);  y = out' @ Wc.T + b.
  Softmax max-subtraction is skipped (|scores| <= 5.7, exp <= 287, safe in
  fp32). Row sums come free as a ones-column in the second matmul.

Sharding: data-parallel over batch: 16 batches -> 8 cores x 2 batches.

Per-core device pipeline:
  phase Q (per batch): DMA x p-major (token s = 32p + t), add theta+pi/2 per
    wire (DVE per-partition scalar), range-reduce mod 2pi, u = ACT Sin,
    13 strided DVE muls -> q9 [128, T, 9] fp32 (col 8 = ones) + fp16 copy,
    PE-transpose chunks -> qT [128, S] fp16 with the 8 feature rows
    replicated at partition strips 0/32/64/96 (for row-group packing).
  phase A (per batch, per 512-token i-block):
    scores: 3 row-group-packed K=8 fp16 matmuls per group -> PSUM [128,1536]
    exp:    one ACT instr per group, scale=1/sqrt2, PSUM->SBUF fp16
    accum:  col-group-packed matmuls X[32s:32s+9] += q9_j^T @ exp
            (strip s = chunk%4; strips summed later by the sel matmul)
    normalize (software-pipelined one i-block behind): DVE copy X->SBUF,
    4 matmuls vs sel[128,9] (sums the 4 strips AND transposes to
    token-major), DVE reciprocal of the ones-row, DVE scale -> osb.
  phase C (per batch, overlaps next batch's attention): the reference's
    swapaxes+reshape+combine is y[128m+p, j] = sum_e oscr[8*(128*mt+p)+e, k]
    * Wc[j,e] + b[j] with m = (S/1024)k + mt: one strided gather DMA into
    glh [9, S] (row 8 = ones for the bias), 32 matmuls vs wcb=[Wc.T; b]
    into one PSUM bank, one DVE copy, one strided store DMA.
"""

import numpy as np

import concourse.bass as bass
import concourse.bacc as bacc
import concourse.tile as tile
from concourse import mybir
from concourse.masks import make_identity
from concourse._compat import with_exitstack

F32 = mybir.dt.float32
F16 = mybir.dt.float16
AF = mybir.ActivationFunctionType
P = 128
E = 8
E9 = 9
IB = 512          # i-block width (tokens per output accumulation block)
JG = 3            # j-chunks per exp group (3 PSUM banks per scores buffer)
INV_SQRT2 = 0.7071067811865476


@with_exitstack
def _body(ctx, tc, x_in, thp, wcb, sel, y, oscr, S, NB):
    nc = tc.nc
    T = S // P                 # token-chunks (tokens per partition)
    NIB = S // IB              # i-blocks per batch
    M4 = S // (P * E)          # row-tiles per combine feature block
    CPI = IB // P              # chunks per i-block (4)

    const = ctx.enter_context(tc.tile_pool(name="const", bufs=1))
    qpool = ctx.enter_context(tc.tile_pool(name="qdata", bufs=1))
    work = ctx.enter_context(tc.tile_pool(name="work", bufs=2))
    expp = ctx.enter_context(tc.tile_pool(name="expp", bufs=3))
    scps = ctx.enter_context(tc.tile_pool(name="scps", bufs=2, space="PSUM"))
    outps = ctx.enter_context(tc.tile_pool(name="outps", bufs=2, space="PSUM"))

    ident = const.tile([P, P], F32)
    make_identity(nc, ident[:])
    thp_sb = const.tile([P, E], F32)
    nc.sync.dma_start(thp_sb[:], thp[:])
    wcb_sb = const.tile([P, E], F16)
    nc.sync.dma_start(wcb_sb[:], wcb[:])
    sel_sb = const.tile([P, E9], F32)
    nc.sync.dma_start(sel_sb[:], sel[:])

    q9 = [qpool.tile([P, T * E9], F32, name=f"q9_{b}") for b in range(NB)]
    q9h = [qpool.tile([P, T * E9], F16, name=f"q9h_{b}") for b in range(NB)]
    qT = [qpool.tile([P, S], F16, name=f"qT_{b}") for b in range(NB)]
    osb = [qpool.tile([P, T * E], F16, name=f"osb_{b}") for b in range(NB)]
    ysb = [qpool.tile([P, T * E], F32, name=f"ysb_{b}") for b in range(NB)]

    # ---------------- phase Q: quantum features --------------------------
    for b in range(NB):
        xb = x_in[b].rearrange("(p t) w -> p (t w)", p=P)
        xs = work.tile([P, T * E], F32, tag="xs")
        nc.sync.dma_start(xs[:], xb)
        x3 = xs.rearrange("p (t w) -> p t w", w=E)
        ph = work.tile([P, T * E], F32, tag="ph")
        p3 = ph.rearrange("p (t w) -> p t w", w=E)
        for w in range(E):
            nc.vector.tensor_scalar_add(p3[:, :, w], x3[:, :, w], thp_sb[:, w : w + 1])
        # range-reduce ph mod 2*pi into [-pi, pi] (Sin spline domain):
        # n = round(ph / 2pi) via the fp32 magic-constant trick, ph -= n * 2pi
        MAGIC = 12582912.0  # 1.5 * 2**23
        TWO_PI = 6.283185307179586
        rt = work.tile([P, T * E], F32, tag="rt")
        nc.vector.tensor_scalar(
            rt[:], ph[:], 1.0 / TWO_PI, MAGIC, mybir.AluOpType.mult, mybir.AluOpType.add
        )
        nc.vector.tensor_scalar(
            rt[:], rt[:], MAGIC, -TWO_PI, mybir.AluOpType.subtract, mybir.AluOpType.mult
        )
        nc.vector.tensor_add(ph[:], ph[:], rt[:])
        us = work.tile([P, T * E], F32, tag="us")
        nc.scalar.activation(us[:], ph[:], AF.Sin)
        u3 = us.rearrange("p (t w) -> p t w", w=E)

        q = q9[b]
        nc.vector.memset(q[:], 1.0)
        q3 = q.rearrange("p (t e) -> p t e", e=E9)
        nc.vector.tensor_mul(q3[:, :, 1], u3[:, :, 0], u3[:, :, 1])
        for w in range(2, E):
            nc.vector.tensor_mul(q3[:, :, w], q3[:, :, w - 1], u3[:, :, w])
        nc.vector.tensor_mul(q3[:, :, 0], u3[:, :, 1], u3[:, :, 2])
        for w in range(3, E):
            nc.vector.tensor_mul(q3[:, :, 0], q3[:, :, 0], u3[:, :, w])

        nc.vector.tensor_copy(q9h[b][:], q[:])
        # transpose q9 token-chunks into qT rows 0:9 (col 128*t + p), then
        # replicate the slice to partition strips 32/64/96 via SBUF DMA
        for c0 in range(0, T, 4):
            tp = outps.tile([P, IB], F32, tag="X")
            for c in range(4):
                nc.tensor.transpose(
                    tp[0:E9, c * P : (c + 1) * P], q3[:, c0 + c, :], ident[:]
                )
            cols = slice(c0 * P, (c0 + 4) * P)
            nc.vector.tensor_copy(qT[b][0:E9, cols], tp[0:E9, :])
            for r in range(1, 4):
                nc.sync.dma_start(qT[b][32 * r : 32 * r + E, cols], qT[b][0:E, cols])

    # ---------------- phases A + C, batch-pipelined -----------------------
    def combine(b):
        # phase C: one gather DMA (row 8 stays ones for the bias),
        # replicate to row strips, 4x row-group-packed matmuls vs wcb.
        glh = qpool.tile([P, M4 * P * E], F16, name=f"glh_{b}")
        nc.vector.memset(glh[:], 1.0)
        glh4 = glh.rearrange("p (mt pp k) -> p mt pp k", pp=P, k=E)
        og = oscr[b].rearrange("(mt pp e) w -> e mt pp w", e=E, pp=P)
        for mt in range(M4):
            nc.sync.dma_start(glh4[0:E, mt], og[:, mt])
        for r in range(1, 4):
            nc.sync.dma_start(glh[32 * r : 32 * r + E, :], glh[0:E, :])
        # 3x row-group-packed matmuls; concurrent outputs go to DIFFERENT
        # PSUM banks (same-bank concurrent drains are fatal on HW)
        rp = scps.tile([P, JG * IB], F32, tag="sc")
        for m in range(S // P):
            k, mt = m // M4, m % M4
            r, c = m % 3, m // 3
            nc.tensor.matmul(
                rp[:, r * IB + c * E : r * IB + (c + 1) * E],
                glh4[32 * r : 32 * r + E9, mt, :, k],
                wcb_sb[32 * r : 32 * r + E9, :],
                start=True,
                stop=True,
                tile_position=(32 * r, 0),
            )
        for m in range(S // P):
            r, c = m % 3, m // 3
            nc.vector.tensor_copy(
                ysb[b][:, m * E : (m + 1) * E],
                rp[:, r * IB + c * E : r * IB + (c + 1) * E],
            )
        nc.sync.dma_start(
            y[b].rearrange("(m pp) j -> pp m j", pp=P),
            ysb[b].rearrange("p (m j) -> p m j", j=E),
        )

    pending_combine = None
    for b in range(NB):
        qh3 = q9h[b].rearrange("p (t e) -> p t e", e=E9)
        o3 = osb[b].rearrange("p (t w) -> p t w", w=E)
        pending = None  # deferred normalize of the previous i-block

        def normalize(X, ib):
            Xs = work.tile([P, IB], F32, tag="Xs")
            nc.vector.tensor_copy(Xs[:], X[:])
            Y = outps.tile([P, IB], F32, tag="X")
            for c in range(CPI):
                nc.tensor.matmul(
                    Y[:, c * E9 : (c + 1) * E9],
                    Xs[:, c * P : (c + 1) * P],
                    sel_sb[:],
                    start=True,
                    stop=True,
                )
            Y3 = Y[:, 0 : CPI * E9].rearrange("p (c e) -> p c e", e=E9)
            rec = work.tile([P, CPI], F32, tag="rec")
            nc.vector.reciprocal(rec[:], Y3[:, :, 8])
            for c in range(CPI):
                nc.vector.tensor_scalar_mul(
                    o3[:, ib * CPI + c, :], Y3[:, c, 0:E], rec[:, c : c + 1]
                )

        for ib in range(NIB):
            X = outps.tile([P, IB], F32, tag="X")
            nc.vector.memset(X[:], 0.0)
            for g0 in range(0, T, JG):
                gn = min(JG, T - g0)
                sc = scps.tile([P, JG * IB], F32, tag="sc")
                for g in range(gn):
                    tj = g0 + g
                    rb = 32 * g
                    nc.tensor.matmul(
                        sc[:, g * IB : (g + 1) * IB],
                        qT[b][rb : rb + E, tj * P : (tj + 1) * P],
                        qT[b][rb : rb + E, ib * IB : (ib + 1) * IB],
                        start=True,
                        stop=True,
                        tile_position=(rb, 0),
                    )
                ex = expp.tile([P, JG * IB], F16, tag="ex")
                nc.scalar.activation(
                    ex[:, 0 : gn * IB], sc[:, 0 : gn * IB], AF.Exp, scale=INV_SQRT2
                )
                for g in range(gn):
                    tj = g0 + g
                    cs = 32 * (tj % 4)
                    nc.tensor.matmul(
                        X[cs : cs + E9, :],
                        qh3[:, tj, :],
                        ex[:, g * IB : (g + 1) * IB],
                        start=(tj == 0),
                        stop=(tj == T - 1),
                        tile_position=(0, cs),
                        skip_group_check=True,
                    )
                if g0 == 0 and pending is not None:
                    normalize(*pending)
                    pending = None
                if g0 == JG and ib == 1 and pending_combine is not None:
                    combine(pending_combine)
                    pending_combine = None
            pending = (X, ib)
        normalize(*pending)
        nc.sync.dma_start(oscr[b].rearrange("(p t) w -> p (t w)", p=P), osb[b][:])
        pending_combine = b
    combine(pending_combine)


def build_nc(S=4096, NB=2):
    nc = bacc.Bacc(None, target_bir_lowering=False)
    x_in = nc.dram_tensor("x", (NB, S, E), F32, kind="ExternalInput")
    thp = nc.dram_tensor("thp", (P, E), F32, kind="ExternalInput")
    wcb = nc.dram_tensor("wcb", (P, E), F16, kind="ExternalInput")
    sel = nc.dram_tensor("sel", (P, E9), F32, kind="ExternalInput")
    y = nc.dram_tensor("y", (NB, S, E), F32, kind="ExternalOutput")
    oscr = nc.dram_tensor("oscr", (NB, S, E), F16)
    with tile.TileContext(nc) as tc:
        _body(tc, x_in[:], thp[:], wcb[:], sel[:], y[:], oscr[:], S, NB)
    nc.compile()
    return nc


def host_inputs(theta, w_combine, b_combine):
    thp = np.tile(
        (np.asarray(theta, np.float32) + np.float32(np.pi / 2))[None, :], (P, 1)
    ).astype(np.float32)
    wcb9 = np.concatenate(
        [np.asarray(w_combine, np.float32).T, np.asarray(b_combine, np.float32)[None]],
        axis=0,
    ).astype(np.float32)
    wcb = np.zeros((P, E), np.float16)
    for st in range(4):
        wcb[32 * st : 32 * st + E9] = wcb9.astype(np.float16)
    sel = np.zeros((P, E9), np.float32)
    for st in range(4):
        for e in range(E9):
            sel[32 * st + e, e] = 1.0
    return thp, wcb, sel


_NC_CACHE = {}


def kernel(x, theta, w_combine, b_combine):
    from concourse.bass_utils import run_bass_kernel_spmd

    x = np.asarray(x, np.float32)
    B, S, _ = x.shape
    NCORES = 8
    NB = B // NCORES
    key = (S, NB)
    if key not in _NC_CACHE:
        _NC_CACHE[key] = build_nc(S=S, NB=NB)
    nc = _NC_CACHE[key]
    thp, wcb, sel = host_inputs(theta, w_combine, b_combine)
    in_maps = [
        {"x": x[c * NB : (c + 1) * NB], "thp": thp, "wcb": wcb, "sel": sel}
        for c in range(NCORES)
    ]
    res = run_bass_kernel_spmd(nc, in_maps, list(range(NCORES))).results
    return np.concatenate([res[c]["y"] for c in range(NCORES)], axis=0)


# revision 18
# speedup vs baseline: 2.1775x; 1.0512x over previous
"""Trainium2 Bass kernel for nn_MultiHeadAttentionQuantum.

Math simplification (verified vs reference to ~5e-7):
  The per-token quantum feature map RX(x+theta) -> CNOT ring -> <Z_w>
  collapses to products of cosines. With u_w = cos(x_w + theta_w):
      q_0 = u1*u2*...*u7
      q_w = u0*u1*...*uw   (w = 1..7)
  Then per batch: scores = q @ q.T / sqrt(2); attn = softmax(scores);
  out = attn @ q; out' = swapaxes(out,1,2).reshape(S,8);  y = out' @ Wc.T + b.
  Softmax max-subtraction is skipped (|scores| <= 5.7, exp <= 287, safe in
  fp32). Row sums come free as a ones-column in the second matmul.

Sharding: data-parallel over batch: 16 batches -> 8 cores x 2 batches.

Per-core device pipeline:
  phase Q (per batch): DMA x p-major (token s = 32p + t), add theta+pi/2 per
    wire (DVE per-partition scalar), range-reduce mod 2pi, u = ACT Sin,
    13 strided DVE muls -> q9 [128, T, 9] fp32 (col 8 = ones) + fp16 copy,
    PE-transpose chunks -> qT [128, S] fp16 with the 8 feature rows
    replicated at partition strips 0/32/64/96 (for row-group packing).
  phase A (per batch, per 512-token i-block):
    scores: 3 row-group-packed K=8 fp16 matmuls per group -> PSUM [128,1536]
    exp:    one ACT instr per group, scale=1/sqrt2, PSUM->SBUF fp16
    accum:  col-group-packed matmuls X[32s:32s+9] += q9_j^T @ exp
            (strip s = chunk%4; strips summed later by the sel matmul)
    normalize (software-pipelined one i-block behind): DVE copy X->SBUF,
    4 matmuls vs sel[128,9] (sums the 4 strips AND transposes to
    token-major), DVE reciprocal of the ones-row, DVE scale -> osb.
  phase C (per batch, overlaps next batch's attention): the reference's
    swapaxes+reshape+combine is y[128m+p, j] = sum_e oscr[8*(128*mt+p)+e, k]
    * Wc[j,e] + b[j] with m = (S/1024)k + mt: one strided gather DMA into
    glh [9, S] (row 8 = ones for the bias), 32 matmuls vs wcb=[Wc.T; b]
    into one PSUM bank, one DVE copy, one strided store DMA.
"""

import numpy as np

import concourse.bass as bass
import concourse.bacc as bacc
import concourse.tile as tile
from concourse import mybir
from concourse.masks import make_identity
from concourse._compat import with_exitstack

F32 = mybir.dt.float32
F16 = mybir.dt.float16
AF = mybir.ActivationFunctionType
P = 128
E = 8
E9 = 9
IB = 512          # i-block width (tokens per output accumulation block)
JG = 3            # j-chunks per exp group (3 PSUM banks per scores buffer)
INV_SQRT2 = 0.7071067811865476


@with_exitstack
def _body(ctx, tc, x_in, thp, wcb, sel, y, oscr, S, NB):
    nc = tc.nc
    T = S // P                 # token-chunks (tokens per partition)
    NIB = S // IB              # i-blocks per batch
    M4 = S // (P * E)          # row-tiles per combine feature block
    CPI = IB // P              # chunks per i-block (4)

    const = ctx.enter_context(tc.tile_pool(name="const", bufs=1))
    qpool = ctx.enter_context(tc.tile_pool(name="qdata", bufs=1))
    work = ctx.enter_context(tc.tile_pool(name="work", bufs=2))
    expp = ctx.enter_context(tc.tile_pool(name="expp", bufs=3))
    scps = ctx.enter_context(tc.tile_pool(name="scps", bufs=2, space="PSUM"))
    outps = ctx.enter_context(tc.tile_pool(name="outps", bufs=2, space="PSUM"))

    ident = const.tile([P, P], F32)
    make_identity(nc, ident[:])
    thp_sb = const.tile([P, E], F32)
    nc.sync.dma_start(thp_sb[:], thp[:])
    wcb_sb = const.tile([P, E], F16)
    nc.sync.dma_start(wcb_sb[:], wcb[:])
    sel_sb = const.tile([P, E9], F32)
    nc.sync.dma_start(sel_sb[:], sel[:])

    q9 = [qpool.tile([P, T * E9], F32, name=f"q9_{b}") for b in range(NB)]
    q9h = [qpool.tile([P, T * E9], F16, name=f"q9h_{b}") for b in range(NB)]
    qT = [qpool.tile([P, S], F16, name=f"qT_{b}") for b in range(NB)]
    osb = [qpool.tile([P, T * E], F16, name=f"osb_{b}") for b in range(NB)]
    ysb = [qpool.tile([P, T * E], F32, name=f"ysb_{b}") for b in range(NB)]

    # ---------------- phase Q: quantum features --------------------------
    for b in range(NB):
        xb = x_in[b].rearrange("(p t) w -> p (t w)", p=P)
        xs = work.tile([P, T * E], F32, tag="xs")
        nc.sync.dma_start(xs[:], xb)
        x3 = xs.rearrange("p (t w) -> p t w", w=E)
        ph = work.tile([P, T * E], F32, tag="ph")
        p3 = ph.rearrange("p (t w) -> p t w", w=E)
        for w in range(E):
            nc.vector.tensor_scalar_add(p3[:, :, w], x3[:, :, w], thp_sb[:, w : w + 1])
        # range-reduce ph mod 2*pi into [-pi, pi] (Sin spline domain):
        # n = round(ph / 2pi) via the fp32 magic-constant trick, ph -= n * 2pi
        MAGIC = 12582912.0  # 1.5 * 2**23
        TWO_PI = 6.283185307179586
        rt = work.tile([P, T * E], F32, tag="rt")
        nc.vector.tensor_scalar(
            rt[:], ph[:], 1.0 / TWO_PI, MAGIC, mybir.AluOpType.mult, mybir.AluOpType.add
        )
        nc.vector.tensor_scalar(
            rt[:], rt[:], MAGIC, -TWO_PI, mybir.AluOpType.subtract, mybir.AluOpType.mult
        )
        nc.vector.tensor_add(ph[:], ph[:], rt[:])
        us = work.tile([P, T * E], F32, tag="us")
        nc.scalar.activation(us[:], ph[:], AF.Sin)
        u3 = us.rearrange("p (t w) -> p t w", w=E)

        q = q9[b]
        nc.vector.memset(q[:], 1.0)
        q3 = q.rearrange("p (t e) -> p t e", e=E9)
        nc.vector.tensor_mul(q3[:, :, 1], u3[:, :, 0], u3[:, :, 1])
        for w in range(2, E):
            nc.vector.tensor_mul(q3[:, :, w], q3[:, :, w - 1], u3[:, :, w])
        nc.vector.tensor_mul(q3[:, :, 0], u3[:, :, 1], u3[:, :, 2])
        for w in range(3, E):
            nc.vector.tensor_mul(q3[:, :, 0], q3[:, :, 0], u3[:, :, w])

        nc.vector.tensor_copy(q9h[b][:], q[:])
        # transpose q9 token-chunks into qT rows 0:9 (col 128*t + p), then
        # replicate the slice to partition strips 32/64/96 via SBUF DMA
        for c0 in range(0, T, 4):
            tp = outps.tile([P, IB], F32, tag="X")
            for c in range(4):
                nc.tensor.transpose(
                    tp[0:E9, c * P : (c + 1) * P], q3[:, c0 + c, :], ident[:]
                )
            cols = slice(c0 * P, (c0 + 4) * P)
            nc.vector.tensor_copy(qT[b][0:E9, cols], tp[0:E9, :])
            for r in range(1, 4):
                nc.sync.dma_start(qT[b][32 * r : 32 * r + E, cols], qT[b][0:E, cols])

    # ---------------- phases A + C, batch-pipelined -----------------------
    def combine(b):
        # phase C: one gather DMA (row 8 stays ones for the bias),
        # replicate to row strips, 4x row-group-packed matmuls vs wcb.
        glh = qpool.tile([P, M4 * P * E], F16, name=f"glh_{b}")
        nc.vector.memset(glh[:], 1.0)
        glh4 = glh.rearrange("p (mt pp k) -> p mt pp k", pp=P, k=E)
        og = oscr[b].rearrange("(mt pp e) w -> e mt pp w", e=E, pp=P)
        for mt in range(M4):
            nc.sync.dma_start(glh4[0:E, mt], og[:, mt])
        # serial fp16 matmuls into one PSUM bank; MM (k, mt) only needs
        # gather piece mt, so matmuls pipeline against the gather DMAs.
        # mt-major order so the first MMs depend on the first piece only.
        rp = scps.tile([P, JG * IB], F32, tag="sc")
        for mi in range(S // P):
            mt, k = mi // E, mi % E
            m = k * M4 + mt
            nc.tensor.matmul(
                rp[:, m * E : (m + 1) * E],
                glh4[0:E9, mt, :, k],
                wcb_sb[0:E9, :],
                start=True,
                stop=True,
            )
        nc.vector.tensor_copy(ysb[b][:], rp[:, 0 : T * E])
        nc.sync.dma_start(
            y[b].rearrange("(m pp) j -> pp m j", pp=P),
            ysb[b].rearrange("p (m j) -> p m j", j=E),
        )

    pending_combine = None
    for b in range(NB):
        qh3 = q9h[b].rearrange("p (t e) -> p t e", e=E9)
        o3 = osb[b].rearrange("p (t w) -> p t w", w=E)
        pending = None  # deferred normalize of the previous i-block

        def normalize(X, ib):
            Xs = work.tile([P, IB], F32, tag="Xs")
            nc.vector.tensor_copy(Xs[:], X[:])
            Y = outps.tile([P, IB], F32, tag="X")
            for c in range(CPI):
                nc.tensor.matmul(
                    Y[:, c * E9 : (c + 1) * E9],
                    Xs[:, c * P : (c + 1) * P],
                    sel_sb[:],
                    start=True,
                    stop=True,
                )
            Y3 = Y[:, 0 : CPI * E9].rearrange("p (c e) -> p c e", e=E9)
            rec = work.tile([P, CPI], F32, tag="rec")
            nc.vector.reciprocal(rec[:], Y3[:, :, 8])
            for c in range(CPI):
                nc.vector.tensor_scalar_mul(
                    o3[:, ib * CPI + c, :], Y3[:, c, 0:E], rec[:, c : c + 1]
                )

        for ib in range(NIB):
            X = outps.tile([P, IB], F32, tag="X")
            nc.vector.memset(X[:], 0.0)
            for g0 in range(0, T, JG):
                gn = min(JG, T - g0)
                sc = scps.tile([P, JG * IB], F32, tag="sc")
                for g in range(gn):
                    tj = g0 + g
                    rb = 32 * g
                    nc.tensor.matmul(
                        sc[:, g * IB : (g + 1) * IB],
                        qT[b][rb : rb + E, tj * P : (tj + 1) * P],
                        qT[b][rb : rb + E, ib * IB : (ib + 1) * IB],
                        start=True,
                        stop=True,
                        tile_position=(rb, 0),
                    )
                ex = expp.tile([P, JG * IB], F16, tag="ex")
                nc.scalar.activation(
                    ex[:, 0 : gn * IB], sc[:, 0 : gn * IB], AF.Exp, scale=INV_SQRT2
                )
                for g in range(gn):
                    tj = g0 + g
                    cs = 32 * (tj % 4)
                    nc.tensor.matmul(
                        X[cs : cs + E9, :],
                        qh3[:, tj, :],
                        ex[:, g * IB : (g + 1) * IB],
                        start=(tj == 0),
                        stop=(tj == T - 1),
                        tile_position=(0, cs),
                        skip_group_check=True,
                    )
                if g0 == 0 and pending is not None:
                    normalize(*pending)
                    pending = None
                if g0 == JG and ib == 1 and pending_combine is not None:
                    combine(pending_combine)
                    pending_combine = None
            pending = (X, ib)
        normalize(*pending)
        nc.sync.dma_start(oscr[b].rearrange("(p t) w -> p (t w)", p=P), osb[b][:])
        pending_combine = b
    combine(pending_combine)


def build_nc(S=4096, NB=2):
    nc = bacc.Bacc(None, target_bir_lowering=False)
    x_in = nc.dram_tensor("x", (NB, S, E), F32, kind="ExternalInput")
    thp = nc.dram_tensor("thp", (P, E), F32, kind="ExternalInput")
    wcb = nc.dram_tensor("wcb", (P, E), F16, kind="ExternalInput")
    sel = nc.dram_tensor("sel", (P, E9), F32, kind="ExternalInput")
    y = nc.dram_tensor("y", (NB, S, E), F32, kind="ExternalOutput")
    oscr = nc.dram_tensor("oscr", (NB, S, E), F16)
    with tile.TileContext(nc) as tc:
        _body(tc, x_in[:], thp[:], wcb[:], sel[:], y[:], oscr[:], S, NB)
    nc.compile()
    return nc


def host_inputs(theta, w_combine, b_combine):
    thp = np.tile(
        (np.asarray(theta, np.float32) + np.float32(np.pi / 2))[None, :], (P, 1)
    ).astype(np.float32)
    wcb9 = np.concatenate(
        [np.asarray(w_combine, np.float32).T, np.asarray(b_combine, np.float32)[None]],
        axis=0,
    ).astype(np.float32)
    wcb = np.zeros((P, E), np.float16)
    for st in range(4):
        wcb[32 * st : 32 * st + E9] = wcb9.astype(np.float16)
    sel = np.zeros((P, E9), np.float32)
    for st in range(4):
        for e in range(E9):
            sel[32 * st + e, e] = 1.0
    return thp, wcb, sel


_NC_CACHE = {}


def kernel(x, theta, w_combine, b_combine):
    from concourse.bass_utils import run_bass_kernel_spmd

    x = np.asarray(x, np.float32)
    B, S, _ = x.shape
    NCORES = 8
    NB = B // NCORES
    key = (S, NB)
    if key not in _NC_CACHE:
        _NC_CACHE[key] = build_nc(S=S, NB=NB)
    nc = _NC_CACHE[key]
    thp, wcb, sel = host_inputs(theta, w_combine, b_combine)
    in_maps = [
        {"x": x[c * NB : (c + 1) * NB], "thp": thp, "wcb": wcb, "sel": sel}
        for c in range(NCORES)
    ]
    res = run_bass_kernel_spmd(nc, in_maps, list(range(NCORES))).results
    return np.concatenate([res[c]["y"] for c in range(NCORES)], axis=0)
